# revision 55
# baseline (speedup 1.0000x reference)
"""Trainium2 Bass kernel for the audio/visual contrastive loss.

Strategy: K-parallel sharding of the visual embedding matmul
E_v = V @ W_v across 8 cores; the tiny audio matmul (K=1280) is fully
replicated per core. Inputs are staged host-side as scaled fp8(e4m3) in a
k-major DoubleRow-interleaved layout.

Schedule: the VISUAL k-stream is DMA'd first so its partials can be cast
and staged as early as possible; audio is DMA'd last and its whole
embed+normalize chain runs hidden under the visual ReduceScatter. PE is
kept continuously busy (p-state ramp) with dependency-free filler matmuls
between the real ones.

Cross-core combine (two collectives):
  1. visual partials are ReduceScattered in fp8 with a sample-interleaved
     chunk layout: core c receives 64 complete reduced visual embeddings
     covering batch pairs [32c, 32c+32) for both v_1 and v_2;
  2. the tail works on RAW (unnormalized) local embeddings: local Grams
     G_vv (64x64) and G_av^T (64 visual x 512 audio) are computed
     immediately, norms come from diag(G_vv), and normalization is applied
     as a per-partition activation scale inside the exp. Numerator terms
     are extracted from exp(G) with per-core 0/1 mask tensors (host-built
     input data, keeping the SPMD program uniform), folded with one SEL
     matmul, and placed into per-core slots of a
     [den(256) | num(256)] payload;
  3. the payload is broadcast 8x and ReduceScattered (replicated-input
     trick = cheap AllReduce): every core receives [sum-den | all num],
     and two Ln+accumulate ops + a subtract yield the loss.
"""

import math
import sys

sys.path.insert(0, "/opt/trn_rl_repo")

import ml_dtypes
import numpy as np

import concourse.bass as bass
import concourse.mybir as mybir
import concourse.tile as tile
from concourse import bacc, bass_utils
from concourse.bass import ts

N_CORES = 8
B = 256          # batch
S = 2 * B        # samples per modality (512)
D = 512          # embedding dim
KV_TOT = 3 * 5 * 48 * 96       # 69120 visual features (lower half)
KV = KV_TOT // N_CORES         # 8640 per core
KVP = 8704                     # padded to 34*256
NT = KVP // 256                # 34 visual double-k-tiles
KA = 1280                      # audio features, replicated per core
NTA = KA // 256                # 5 audio double-k-tiles
SA = S + 64                    # audio cols: 512 canonical + 64 per-core extra
SX = 16.0                      # fp8 scale for activations
SW = 256.0                     # fp8 scale for weights
SP8 = 1.0 / 128.0              # payload scale: the REDUCED sum must fit fp8
SXN = 16.0                     # fp8 scale for normalized audio embeddings

# filler-matmul batch sizes (tuned against the timeline sim)
N_JS = 0         # fillers after each k-stream chunk (p-state keep-alive)
N_JA = 30        # after the visual casts, before the audio matmuls
N_JB = 18        # inside the audio chain (pauxp scope)
N_JC = 38        # tail-pool bridge through the ReduceScatter window

# mask tensor column layout (per-core [64, 992] bf16 input)
MK_I64 = 0       # [64,64] identity        (diag of G_vv -> norms)
MK_PSH = 64      # [64,64] +32 shift perm  (rn[(s+32)%64])
MK_M32 = 128     # [64,64] pair mask *SXN^2 (G_vv[j,32+j], j<32)
MK_SEL = 192     # [64,32] fold rows j,j+32 -> batch j
MK_PLC = 224     # [32,256] place local batch into global cols
MK_MAV = 480     # [64,512] pick cols 32c+(j%32) of both audio halves
MK_N = 992

F32 = mybir.dt.float32
F8 = mybir.dt.float8e4
BF16 = mybir.dt.bfloat16
AF = mybir.ActivationFunctionType
DR = mybir.MatmulPerfMode.DoubleRow
ALU = mybir.AluOpType

_CACHE = {}


def build():
    nc = bacc.Bacc("TRN2", target_bir_lowering=False, debug=False,
                   num_devices=N_CORES)

    xv_d = nc.dram_tensor("xv", [128, NT * 2 * S], F8, kind="ExternalInput")
    wv_d = nc.dram_tensor("wv", [128, NT * 2 * D], F8, kind="ExternalInput")
    xa_d = nc.dram_tensor("xa", [128, NTA * 2 * SA], F8, kind="ExternalInput")
    wa_d = nc.dram_tensor("wa", [128, NTA * 2 * D], F8, kind="ExternalInput")
    mk_d = nc.dram_tensor("mk", [64, MK_N], BF16, kind="ExternalInput")
    loss_d = nc.dram_tensor("loss", [1, 1], F32, kind="ExternalOutput")

    # last chunks are 1 tile so the PE trail after the DMA stream is short
    chunks = [(0, 4), (4, 8), (8, 12), (12, 16), (16, 20), (20, 24),
              (24, 28), (28, 31), (31, 33), (33, 34)]

    with tile.TileContext(nc) as tc:
        with tc.tile_pool(name="const", bufs=1) as constp, \
             tc.tile_pool(name="emb", bufs=1) as embp, \
             tc.tile_pool(name="dram", bufs=1, space="DRAM") as dramp:
            ones_bf = constp.tile([128, 1], BF16)
            nc.vector.memset(ones_bf[:], 1.0)
            ones_row_bf = constp.tile([1, 128], BF16)
            nc.vector.memset(ones_row_bf[:], 1.0)
            bias_n30 = constp.tile([64, 1], F32)
            nc.vector.memset(bias_n30[0:32, :], 0.0)
            nc.vector.memset(bias_n30[32:64, :], -30.0)
            bias_pl16 = constp.tile([128, 1], F32)
            nc.vector.memset(bias_pl16[:], float(math.log(SXN)))
            bias_nl16 = constp.tile([64, 1], F32)
            nc.vector.memset(bias_nl16[:], float(-math.log(SXN)))
            from concourse.hw_specs import get_activation_tables
            tables = list(get_activation_tables(nc.m.arch))
            joint_id = tables.index("natural_log_exp_and_others")
            nc.scalar.add_instruction(
                mybir.InstLoadActFuncSet(
                    name=nc.get_next_instruction_name(),
                    ins=[], outs=[], act_func_set_id=joint_id))

            # per-core mask constants (also the filler-matmul moving
            # operand, so fillers depend on nothing else)
            mk = embp.tile([64, MK_N], BF16)
            nc.sync.dma_start(out=mk[:], in_=mk_d.ap())

            er_n8 = embp.tile([128, 4, S], F8)       # normalized audio (xSXN)
            a6p = embp.tile([32, 1], BF16)           # exp(a1*a2), local batch
            # scaled visual partials, chunk-major for the RS staging DMA
            e8v = embp.tile([128, N_CORES, 4, 64], F8)
            in_b = dramp.tile([N_CORES * 4 * 128, 64], F8)
            rs_b = dramp.tile([4 * 128, 64], F8)
            rs2_in = dramp.tile([8, 512], BF16)
            rs2_out = dramp.tile([1, 512], BF16)

            xr = xv_d.ap().rearrange("p (t i n) -> p t i n", t=NT, i=2)
            wr = wv_d.ap().rearrange("p (t i n) -> p t i n", t=NT, i=2)
            xar = xa_d.ap().rearrange("p (t i n) -> p t i n", t=NTA, i=2)
            war = wa_d.ap().rearrange("p (t i n) -> p t i n", t=NTA, i=2)

            with tc.tile_pool(name="xin", bufs=1) as xinp:
                # ---- visual k-stream DMAs FIRST (RS1 start gates on them) --
                xc, wc = [], []
                for g, (t0, t1) in enumerate(chunks):
                    w_g = xinp.tile([128, t1 - t0, 2, D], F8, tag=f"wc{g}")
                    nc.sync.dma_start(out=w_g[:], in_=wr[:, t0:t1])
                    x_g = xinp.tile([128, t1 - t0, 2, S], F8, tag=f"xc{g}")
                    nc.sync.dma_start(out=x_g[:], in_=xr[:, t0:t1])
                    xc.append(x_g)
                    wc.append(w_g)
                # audio tiles (DMAs issued after the partial-staging DMAs so
                # the staging transfers are not queued behind them)
                xa_sb = xinp.tile([128, NTA, 2, SA], F8, tag="xa")
                wa_sb = xinp.tile([128, NTA, 2, D], F8, tag="wa")

                with tc.tile_pool(name="pacc", bufs=1, space="PSUM") as paccp:
                    psum_v = [paccp.tile([128, S], F32, tag=f"pv{d}",
                                         name=f"psum_v{d}") for d in range(4)]
                    junk_s = paccp.tile([1, 512], F32, tag="jks")
                    tmap = {}
                    for g, (t0, t1) in enumerate(chunks):
                        for t in range(t0, t1):
                            tmap[t] = (g, t - t0)
                    fill_after = {t1 - 1 for _, t1 in chunks[:6]}
                    for t in range(NT):
                        g, r = tmap[t]
                        for d in range(4):
                            nc.tensor.matmul(psum_v[d][:],
                                             wc[g][:, r, :, ts(d, 128)],
                                             xc[g][:, r],
                                             start=(t == 0),
                                             stop=(t == NT - 1),
                                             perf_mode=DR)
                        # keep the PE p-state ramp alive across the
                        # DMA-paced chunk boundaries
                        if t in fill_after:
                            for _ in range(N_JS):
                                nc.tensor.matmul(junk_s[:, 0:256],
                                                 ones_bf[0:64, :],
                                                 mk[:, 0:256],
                                                 start=True, stop=True,
                                                 skip_group_check=True)
                    # scaled fp8 payload, staged in the sample-interleaved
                    # ReduceScatter chunk layout; two halves so the first
                    # stage DMA overlaps the second casts
                    in_v = in_b[:].rearrange("(c p d) u -> p c (d u)",
                                             c=N_CORES, d=4, p=128)
                    e8r = e8v[:].rearrange("p c d u -> p c (d u)")
                    for dp in range(2):
                        for d in (2 * dp, 2 * dp + 1):
                            src = psum_v[d][:].rearrange(
                                "p (c u) -> p c u", c=N_CORES)
                            if d % 2 == 1:
                                nc.vector.tensor_scalar_mul(
                                    e8v[:, :, d, :], src, SP8)
                            else:
                                nc.scalar.activation(e8v[:, :, d, :], src,
                                                     AF.Copy, scale=SP8)
                        du = slice(dp * 128, dp * 128 + 128)
                        nc.sync.dma_start(out=in_v[:, :, du],
                                          in_=e8r[:, :, du])
                    # per-tile pieces: short transfers interleave with the
                    # cast-gated partial-staging DMAs on the DMA engines
                    for t in range(NTA):
                        nc.sync.dma_start(out=xa_sb[:, t], in_=xar[:, t])
                        nc.sync.dma_start(out=wa_sb[:, t], in_=war[:, t])
                    # the visual ReduceScatter + result load go on the Pool
                    # queue NOW so nothing later delays their dispatch;
                    # er8 loads via DVE-issued DMA so the in-order SP queue
                    # (audio DMAs behind it) is not blocked on RS1
                    nc.gpsimd.collective_compute(
                        "ReduceScatter", ALU.add,
                        replica_groups=[list(range(N_CORES))],
                        ins=[in_b[:]], outs=[rs_b[:]],
                    )
                    er8 = embp.tile([128, 4, 64], F8)
                    nc.sync.dma_start(
                        out=er8[:],
                        in_=rs_b[:].rearrange("(p d) u -> p d u", p=128))
                    # filler batch A: bridge the PE gap between the k-stream
                    # trail and the audio matmuls (dead psum region, mk input)
                    for _ in range(N_JA):
                        nc.tensor.matmul(junk_s[:],
                                         ones_bf[0:64, :], mk[:, 0:512],
                                         start=True, stop=True)

                # ---- audio embed + normalize (hidden under RS1) ----
                e_a = embp.tile([128, 4, S], BF16)
                e_ax = embp.tile([128, 4, 64], BF16)
                with tc.tile_pool(name="pau", bufs=1, space="PSUM") as paup:
                    psum_a = [paup.tile([128, S], F32, tag=f"pa{d}",
                                        name=f"psum_a{d}") for d in range(4)]
                    psum_ax = [paup.tile([128, 64], F32, tag=f"px{d}",
                                         name=f"psum_ax{d}") for d in range(4)]
                    for t in range(NTA):
                        for d in range(4):
                            nc.tensor.matmul(psum_a[d][:],
                                             wa_sb[:, t, :, ts(d, 128)],
                                             xa_sb[:, t, :, 0:S],
                                             start=(t == 0),
                                             stop=(t == NTA - 1),
                                             perf_mode=DR)
                            nc.tensor.matmul(psum_ax[d][:],
                                             wa_sb[:, t, :, ts(d, 128)],
                                             xa_sb[:, t, :, S:SA],
                                             start=(t == 0),
                                             stop=(t == NTA - 1),
                                             perf_mode=DR)
                    for d in range(4):
                        if d < 2:
                            nc.vector.tensor_copy(e_a[:, d], psum_a[d][:])
                            nc.vector.tensor_copy(e_ax[:, d], psum_ax[d][:])
                        else:
                            nc.scalar.copy(e_a[:, d], psum_a[d][:])
                            nc.scalar.copy(e_ax[:, d], psum_ax[d][:])

                with tc.tile_pool(name="paux", bufs=1, space="PSUM") as pauxp:
                    junk_b = pauxp.tile([1, 512], F32, tag="jb")
                    for _ in range(N_JB // 3):
                        nc.tensor.matmul(junk_b[:], ones_bf[0:64, :],
                                         mk[:, 0:512], start=True, stop=True)
                    sq_a = embp.tile([128, 4, SA], BF16)
                    nc.vector.tensor_mul(sq_a[:, :, 0:S], e_a[:], e_a[:])
                    nc.vector.tensor_mul(sq_a[:, :, S:SA], e_ax[:], e_ax[:])
                    psh_a = pauxp.tile([1, SA], F32, tag="psha")
                    for d in range(4):
                        nc.tensor.matmul(psh_a[:, 0:S], ones_bf[:],
                                         sq_a[:, d, 0:S],
                                         start=(d == 0), stop=(d == 3))
                    for d in range(4):
                        nc.tensor.matmul(psh_a[:, S:SA], ones_bf[:],
                                         sq_a[:, d, S:SA],
                                         start=(d == 0), stop=(d == 3))
                    for _ in range(N_JB // 3):
                        nc.tensor.matmul(junk_b[:], ones_bf[0:64, :],
                                         mk[:, 0:512], start=True, stop=True)
                    ln_a = embp.tile([1, SA], BF16)
                    nc.scalar.activation(ln_a[:], psh_a[:], AF.Ln)
                    lnb_a = pauxp.tile([128, S], F32, tag="lnb")
                    nc.tensor.matmul(lnb_a[:], ones_row_bf[:], ln_a[0:1, 0:S],
                                     start=True, stop=True)
                    # rn_a16 = SXN * |e_a|^-1 (joint table: exp(-.5 ln + ln16))
                    rn_a16 = embp.tile([128, S], BF16)
                    nc.scalar.activation(rn_a16[:], lnb_a[:], AF.Exp,
                                         scale=-0.5, bias=bias_pl16[:])
                    for d in range(4):
                        nc.vector.tensor_mul(er_n8[:, d], e_a[:, d],
                                             rn_a16[:])
                    lnb_x = pauxp.tile([128, S], F32, tag="lnb")
                    nc.tensor.matmul(lnb_x[:, 0:64], ones_row_bf[:],
                                     ln_a[0:1, S:SA], start=True, stop=True)
                    rn_x = embp.tile([128, 64], BF16)
                    nc.scalar.activation(rn_x[:], lnb_x[:, 0:64], AF.Exp,
                                         scale=-0.5)
                    er_nx = embp.tile([128, 4, 64], BF16)
                    for d in range(4):
                        nc.vector.tensor_mul(er_nx[:, d], e_ax[:, d], rn_x[:])
                    # local a1*a2 diagonal dots -> [32,1] partition layout
                    tpa = embp.tile([128, 4, 32], BF16)
                    nc.vector.tensor_mul(tpa[:], er_nx[:, :, 0:32],
                                         er_nx[:, :, 32:64])
                    pa6 = pauxp.tile([32, 1], F32, tag="pa6")
                    for d in range(4):
                        nc.tensor.matmul(pa6[:], tpa[:, d, :],
                                         ones_bf[:, 0:1],
                                         start=(d == 0), stop=(d == 3))
                    nc.scalar.activation(a6p[:], pa6[:], AF.Exp)
                    for _ in range(N_JB - 2 * (N_JB // 3)):
                        nc.tensor.matmul(junk_b[:], ones_bf[0:64, :],
                                         mk[:, 0:512], start=True, stop=True)

            # ---------------- local tail on raw chunk embeddings ---------
            if True:
                with tc.tile_pool(name="tail", bufs=1) as tp:
                  with tc.tile_pool(name="pmid", bufs=1, space="PSUM") as pm:
                    # tail-pool filler bridge: keeps the PE engine busy (and
                    # its p-state ramped) through the ReduceScatter + er8
                    # load so the Gram matmuls below run at full clock
                    junk_t = pm.tile([1, 512], F32, tag="jkt")
                    for _ in range(N_JC):
                        nc.tensor.matmul(junk_t[:], ones_bf[0:64, :],
                                         mk[:, 0:512], start=True, stop=True)

                    # contrib columns: {a1-v exps, a2-v exps, v1v2, a1a2}
                    contrib = tp.tile([64, 4], BF16)
                    nc.vector.memset(contrib[:, 3:4], 0.0)
                    nc.vector.tensor_copy(contrib[0:32, 3:4], a6p[:])

                    # local Grams on raw embeddings
                    gvv = pm.tile([64, 64], F32, tag="gvv")
                    for d in range(4):
                        nc.tensor.matmul(gvv[:], er8[:, d, :], er8[:, d, :],
                                         start=(d == 0), stop=(d == 3))
                    gav = pm.tile([64, S], F32, tag="gav")
                    for d in range(4):
                        nc.tensor.matmul(gav[:], er8[:, d, :], er_n8[:, d, :],
                                         start=(d == 0), stop=(d == 3))

                    # norms from diag(G_vv); rn = |E|^-1 / SXN
                    junk_a = tp.tile([64, 64], BF16)
                    nsq = tp.tile([64, 1], F32)
                    nc.vector.tensor_mul(junk_a[:], gvv[:],
                                         mk[:, MK_I64:MK_I64 + 64])
                    nc.vector.reduce_sum(nsq[:], junk_a[:],
                                         axis=mybir.AxisListType.X)
                    # raw v1*v2 pair dots (M32 already carries SXN^2);
                    # emitted before the rn_bm copy so the in-order DVE queue
                    # is not parked behind the rn_f dependency
                    junk_bb = tp.tile([64, 64], BF16)
                    dvv = tp.tile([64, 1], F32)
                    nc.vector.tensor_mul(junk_bb[:], gvv[:],
                                         mk[:, MK_M32:MK_M32 + 64])
                    nc.vector.reduce_sum(dvv[:], junk_bb[:],
                                         axis=mybir.AxisListType.X)
                    lns = tp.tile([64, 1], F32)
                    nc.scalar.activation(lns[:], nsq[:], AF.Ln)
                    rn_f = tp.tile([64, 1], F32)
                    nc.scalar.activation(rn_f[:], lns[:], AF.Exp,
                                         scale=-0.5, bias=bias_nl16[:])
                    rn_bm = tp.tile([64, 1], BF16)
                    nc.vector.tensor_copy(rn_bm[:], rn_f[:])
                    psh_p = pm.tile([64, 1], F32, tag="pshp")
                    nc.tensor.matmul(psh_p[:], mk[:, MK_PSH:MK_PSH + 64],
                                     rn_bm[:], start=True, stop=True)
                    rnp12 = tp.tile([64, 1], F32)
                    nc.vector.tensor_mul(rnp12[:], rn_f[:], psh_p[:])

                    # exp Gram with normalization folded into the act scale
                    exp_gt = tp.tile([64, S], BF16)
                    junk_c = tp.tile([64, S], BF16)
                    denp = pm.tile([1, 256], F32, tag="denp")
                    nc.scalar.activation(exp_gt[:], gav[:], AF.Exp,
                                         scale=rn_f[:])
                    nc.scalar.activation(contrib[:, 2:3], dvv[:], AF.Exp,
                                         scale=rnp12[:], bias=bias_n30[:])
                    nc.vector.tensor_mul(junk_c[:], exp_gt[:],
                                         mk[:, MK_MAV:MK_MAV + S])
                    nc.tensor.matmul(denp[:], ones_bf[0:64, :],
                                     exp_gt[:, 0:256], start=True, stop=False)
                    nc.tensor.matmul(denp[:], ones_bf[0:64, :],
                                     exp_gt[:, 256:512], start=False,
                                     stop=True)
                    with nc.allow_low_precision(
                            reason="sum of exps in bf16; tolerance 2e-2"):
                        nc.vector.reduce_sum(
                            contrib[:, 0:2],
                            junk_c[:].rearrange("p (h n) -> p h n", h=2),
                            axis=mybir.AxisListType.X)

                    # fold rows j/j+32; place num into global batch cols
                    selp = pm.tile([32, 4], F32, tag="selp")
                    nc.tensor.matmul(selp[:], mk[:, MK_SEL:MK_SEL + 32],
                                     contrib[:], start=True, stop=True)
                    nsum = tp.tile([32, 1], F32)
                    nc.vector.reduce_sum(nsum[:], selp[:],
                                         axis=mybir.AxisListType.X)
                    recip = tp.tile([32, 1], BF16)
                    with nc.allow_low_precision(
                            reason="1/num in bf16; tolerance 2e-2"):
                        nc.vector.reciprocal(recip[:], nsum[:])
                    placep = pm.tile([1, 256], F32, tag="plcp")
                    nc.tensor.matmul(placep[:], recip[:],
                                     mk[0:32, MK_PLC:MK_PLC + 256],
                                     start=True, stop=True)

                    # payload = [den partial (256) | num in our slot (256)];
                    # stage 8 replicas with one broadcast-read DMA
                    payload = tp.tile([1, 512], BF16)
                    nc.scalar.copy(payload[:, 0:256], denp[:])
                    nc.vector.tensor_copy(payload[:, 256:512], placep[:])
                    nc.sync.dma_start(
                        out=rs2_in[:].rearrange("(o a) b -> o (a b)", o=1),
                        in_=payload[:].rearrange(
                            "p (o n) -> p o n", o=1).to_broadcast((1, 8, 512)))

                  # (pmid closed: the final reduction gets its own psum)
                  if True:
                    # replicated-input ReduceScatter == cheap AllReduce:
                    # every core receives [sum-den(256) | all num(256)]
                    nc.gpsimd.collective_compute(
                        "ReduceScatter", ALU.add,
                        replica_groups=[list(range(N_CORES))],
                        ins=[rs2_in[:]], outs=[rs2_out[:]],
                    )
                    g2 = tp.tile([1, 512], BF16)
                    nc.sync.dma_start(out=g2[:], in_=rs2_out[:].opt())
                    # one Ln over [den | 1/num]: accum = sum ln den - sum ln
                    # num = 256 * loss
                    l_all = tp.tile([1, 512], F32)
                    dsum = tp.tile([1, 1], F32)
                    nc.scalar.activation(l_all[:], g2[:], AF.Ln,
                                         accum_out=dsum[:])
                    loss_sb = tp.tile([1, 1], F32)
                    nc.scalar.activation(loss_sb[:], dsum[:], AF.Copy,
                                         scale=float(1.0 / B))
                    nc.sync.dma_start(out=loss_d.ap(), in_=loss_sb[:])

    nc.compile()
    return nc


def _get_nc():
    if "nc" not in _CACHE:
        _CACHE["nc"] = build()
    return _CACHE["nc"]


def _dr_layout(m, nt):
    """[nt*256, N] k-major -> [128, nt*2*N] DoubleRow DMA layout.
    Logical k = t*256 + i*128 + p lands at [p, t, i, :]."""
    n = m.shape[1]
    return np.ascontiguousarray(
        m.reshape(nt, 2, 128, n).transpose(2, 0, 1, 3)).reshape(128, nt * 2 * n)


def _vperm():
    """Permuted visual sample order: chunk c = [v1 batch 32c..32c+32,
    v2 batch 32c..32c+32]; v2 originals live at sample index 256+i."""
    perm = []
    for c in range(N_CORES):
        perm.extend(range(32 * c, 32 * c + 32))
        perm.extend(range(256 + 32 * c, 256 + 32 * c + 32))
    return np.asarray(perm)


def _masks(c):
    """Per-core [64, MK_N] mask constants (see MK_* layout)."""
    mk = np.zeros((64, MK_N), np.float32)
    r = np.arange(64)
    mk[r, MK_I64 + r] = 1.0                        # identity
    s = np.arange(64)
    mk[(s + 32) % 64, MK_PSH + s] = 1.0            # +32 shift permutation
    j = np.arange(32)
    mk[j, MK_M32 + 32 + j] = SXN * SXN             # v1-v2 pair mask
    mk[j, MK_SEL + j] = 1.0                        # fold rows j, j+32
    mk[j + 32, MK_SEL + j] = 1.0
    mk[j, MK_PLC + 32 * c + j] = 1.0               # place into global cols
    col = 32 * c + (r % 32)
    mk[r, MK_MAV + col] = 1.0                      # a1 block
    mk[r, MK_MAV + 256 + col] = 1.0                # a2 block
    return mk.astype(ml_dtypes.bfloat16)


def _shard_inputs(a_1, v_1, a_2, v_2, W_a, W_v):
    f8 = ml_dtypes.float8_e4m3
    A = np.concatenate([a_1, a_2], axis=0).reshape(S, KA)
    V = np.concatenate([v_1, v_2], axis=0)
    V = V.reshape(S, 15, 96, 96)[:, :, 48:, :].reshape(S, KV_TOT)
    Wvp = np.ascontiguousarray(
        W_v.reshape(5, 3, 48 * 96, D).transpose(1, 0, 2, 3)
    ).reshape(KV_TOT, D)

    A8 = (A.T * SX).astype(f8)                 # (1280, 512)
    V8 = (V.T * SX).astype(f8)[:, _vperm()]    # (69120, 512) permuted cols
    Wa8 = (W_a * SW).astype(f8)
    Wv8 = (Wvp * SW).astype(f8)

    wa = _dr_layout(np.ascontiguousarray(Wa8), NTA)

    in_maps = []
    for c in range(N_CORES):
        xv = np.zeros((KVP, S), f8)
        xv[:KV] = V8[c * KV:(c + 1) * KV]
        wv = np.zeros((KVP, D), f8)
        wv[:KV] = Wv8[c * KV:(c + 1) * KV]
        # canonical audio + this core's 64 pair columns (a1 then a2)
        ec = list(range(32 * c, 32 * c + 32)) + \
             list(range(256 + 32 * c, 256 + 32 * c + 32))
        xa_c = np.concatenate([A8, A8[:, ec]], axis=1)   # (1280, 576)
        in_maps.append({
            "xv": _dr_layout(xv, NT),
            "wv": _dr_layout(wv, NT),
            "xa": _dr_layout(np.ascontiguousarray(xa_c), NTA),
            "wa": wa,
            "mk": _masks(c),
        })
    return in_maps


def kernel(a_1, v_1, a_2, v_2, W_a, W_v):
    nc = _get_nc()
    in_maps = _shard_inputs(np.asarray(a_1, np.float32),
                            np.asarray(v_1, np.float32),
                            np.asarray(a_2, np.float32),
                            np.asarray(v_2, np.float32),
                            np.asarray(W_a, np.float32),
                            np.asarray(W_v, np.float32))
    res = bass_utils.run_bass_kernel_spmd(nc, in_maps,
                                          core_ids=list(range(N_CORES)))
    return np.asarray(res.results[0]["loss"], np.float32).reshape(())


# revision 56
# speedup vs baseline: 1.0156x; 1.0156x over previous
"""Trainium2 Bass kernel for the audio/visual contrastive loss.

Strategy: K-parallel sharding of the visual embedding matmul
E_v = V @ W_v across 8 cores; the tiny audio matmul (K=1280) is fully
replicated per core. Inputs are staged host-side as scaled fp8(e4m3) in a
k-major DoubleRow-interleaved layout.

Schedule: the VISUAL k-stream is DMA'd first so its partials can be cast
and staged as early as possible; audio is DMA'd last and its whole
embed+normalize chain runs hidden under the visual ReduceScatter. PE is
kept continuously busy (p-state ramp) with dependency-free filler matmuls
between the real ones.

Cross-core combine (two collectives):
  1. visual partials are ReduceScattered in fp8 with a sample-interleaved
     chunk layout: core c receives 64 complete reduced visual embeddings
     covering batch pairs [32c, 32c+32) for both v_1 and v_2;
  2. the tail works on RAW (unnormalized) local embeddings: local Grams
     G_vv (64x64) and G_av^T (64 visual x 512 audio) are computed
     immediately, norms come from diag(G_vv), and normalization is applied
     as a per-partition activation scale inside the exp. Numerator terms
     are extracted from exp(G) with per-core 0/1 mask tensors (host-built
     input data, keeping the SPMD program uniform), folded with one SEL
     matmul, and placed into per-core slots of a
     [den(256) | num(256)] payload;
  3. the payload is broadcast 8x and ReduceScattered (replicated-input
     trick = cheap AllReduce): every core receives [sum-den | all num],
     and two Ln+accumulate ops + a subtract yield the loss.
"""

import math
import sys

sys.path.insert(0, "/opt/trn_rl_repo")

import ml_dtypes
import numpy as np

import concourse.bass as bass
import concourse.mybir as mybir
import concourse.tile as tile
from concourse import bacc, bass_utils
from concourse.bass import ts

N_CORES = 8
B = 256          # batch
S = 2 * B        # samples per modality (512)
D = 512          # embedding dim
KV_TOT = 3 * 5 * 48 * 96       # 69120 visual features (lower half)
KV = KV_TOT // N_CORES         # 8640 per core
KVP = 8704                     # padded to 34*256
NT = KVP // 256                # 34 visual double-k-tiles
KA = 1280                      # audio features, replicated per core
NTA = KA // 256                # 5 audio double-k-tiles
SA = S + 64                    # audio cols: 512 canonical + 64 per-core extra
SX = 16.0                      # fp8 scale for activations
SW = 256.0                     # fp8 scale for weights
SP8 = 1.0 / 128.0              # payload scale: the REDUCED sum must fit fp8
SXN = 16.0                     # fp8 scale for normalized audio embeddings

# filler-matmul batch sizes (tuned against the timeline sim)
N_JS = 0         # fillers after each k-stream chunk (p-state keep-alive)
N_JA = 30        # after the visual casts, before the audio matmuls
N_JB = 18        # inside the audio chain (pauxp scope)
N_JC = 38        # tail-pool bridge through the ReduceScatter window

# mask tensor column layout (per-core [64, 992] bf16 input)
MK_I64 = 0       # [64,64] identity        (diag of G_vv -> norms)
MK_PSH = 64      # [64,64] +32 shift perm  (rn[(s+32)%64])
MK_M32 = 128     # [64,64] pair mask *SXN^2 (G_vv[j,32+j], j<32)
MK_SEL = 192     # [64,32] fold rows j,j+32 -> batch j
MK_PLC = 224     # [32,256] place local batch into global cols
MK_MAV = 480     # [64,512] pick cols 32c+(j%32) of both audio halves
MK_N = 992

F32 = mybir.dt.float32
F8 = mybir.dt.float8e4
BF16 = mybir.dt.bfloat16
AF = mybir.ActivationFunctionType
DR = mybir.MatmulPerfMode.DoubleRow
ALU = mybir.AluOpType

_CACHE = {}


def build():
    nc = bacc.Bacc("TRN2", target_bir_lowering=False, debug=False,
                   num_devices=N_CORES)

    xv_d = nc.dram_tensor("xv", [128, NT * 2 * S], F8, kind="ExternalInput")
    wv_d = nc.dram_tensor("wv", [128, NT * 2 * D], F8, kind="ExternalInput")
    xa_d = nc.dram_tensor("xa", [128, NTA * 2 * SA], F8, kind="ExternalInput")
    wa_d = nc.dram_tensor("wa", [128, NTA * 2 * D], F8, kind="ExternalInput")
    mk_d = nc.dram_tensor("mk", [64, MK_N], BF16, kind="ExternalInput")
    loss_d = nc.dram_tensor("loss", [1, 1], F32, kind="ExternalOutput")

    # last chunks are 1 tile so the PE trail after the DMA stream is short
    chunks = [(0, 4), (4, 8), (8, 12), (12, 16), (16, 20), (20, 24),
              (24, 28), (28, 31), (31, 33), (33, 34)]

    with tile.TileContext(nc) as tc:
        with tc.tile_pool(name="const", bufs=1) as constp, \
             tc.tile_pool(name="emb", bufs=1) as embp, \
             tc.tile_pool(name="dram", bufs=1, space="DRAM") as dramp:
            ones_bf = constp.tile([128, 1], BF16)
            nc.vector.memset(ones_bf[:], 1.0)
            ones_row_bf = constp.tile([1, 128], BF16)
            nc.vector.memset(ones_row_bf[:], 1.0)
            bias_n30 = constp.tile([64, 1], F32)
            nc.vector.memset(bias_n30[0:32, :], 0.0)
            nc.vector.memset(bias_n30[32:64, :], -30.0)
            bias_pl16 = constp.tile([128, 1], F32)
            nc.vector.memset(bias_pl16[:], float(math.log(SXN)))
            bias_nl16 = constp.tile([64, 1], F32)
            nc.vector.memset(bias_nl16[:], float(-math.log(SXN)))
            from concourse.hw_specs import get_activation_tables
            tables = list(get_activation_tables(nc.m.arch))
            joint_id = tables.index("natural_log_exp_and_others")
            nc.scalar.add_instruction(
                mybir.InstLoadActFuncSet(
                    name=nc.get_next_instruction_name(),
                    ins=[], outs=[], act_func_set_id=joint_id))

            # per-core mask constants (also the filler-matmul moving
            # operand, so fillers depend on nothing else)
            mk = embp.tile([64, MK_N], BF16)
            nc.sync.dma_start(out=mk[:], in_=mk_d.ap())

            er_n8 = embp.tile([128, 4, S], F8)       # normalized audio (xSXN)
            a6p = embp.tile([32, 1], BF16)           # exp(a1*a2), local batch
            # scaled visual partials, chunk-major for the RS staging DMA
            e8v = embp.tile([128, N_CORES, 4, 64], F8)
            in_b = dramp.tile([N_CORES * 4 * 128, 64], F8)
            rs_b = dramp.tile([4 * 128, 64], F8)
            rs2_in = dramp.tile([8, 512], BF16)
            rs2_out = dramp.tile([1, 512], BF16)

            xr = xv_d.ap().rearrange("p (t i n) -> p t i n", t=NT, i=2)
            wr = wv_d.ap().rearrange("p (t i n) -> p t i n", t=NT, i=2)
            xar = xa_d.ap().rearrange("p (t i n) -> p t i n", t=NTA, i=2)
            war = wa_d.ap().rearrange("p (t i n) -> p t i n", t=NTA, i=2)

            with tc.tile_pool(name="xin", bufs=1) as xinp:
                # ---- visual k-stream DMAs FIRST (RS1 start gates on them) --
                xc, wc = [], []
                for g, (t0, t1) in enumerate(chunks):
                    w_g = xinp.tile([128, t1 - t0, 2, D], F8, tag=f"wc{g}")
                    nc.sync.dma_start(out=w_g[:], in_=wr[:, t0:t1])
                    x_g = xinp.tile([128, t1 - t0, 2, S], F8, tag=f"xc{g}")
                    nc.sync.dma_start(out=x_g[:], in_=xr[:, t0:t1])
                    xc.append(x_g)
                    wc.append(w_g)
                # audio tiles (DMAs issued after the partial-staging DMAs so
                # the staging transfers are not queued behind them)
                xa_sb = xinp.tile([128, NTA, 2, SA], F8, tag="xa")
                wa_sb = xinp.tile([128, NTA, 2, D], F8, tag="wa")

                with tc.tile_pool(name="pacc", bufs=1, space="PSUM") as paccp:
                    psum_v = [paccp.tile([128, S], F32, tag=f"pv{d}",
                                         name=f"psum_v{d}") for d in range(4)]
                    junk_s = paccp.tile([1, 512], F32, tag="jks")
                    tmap = {}
                    for g, (t0, t1) in enumerate(chunks):
                        for t in range(t0, t1):
                            tmap[t] = (g, t - t0)
                    fill_after = {t1 - 1 for _, t1 in chunks[:6]}
                    for t in range(NT):
                        g, r = tmap[t]
                        for d in range(4):
                            nc.tensor.matmul(psum_v[d][:],
                                             wc[g][:, r, :, ts(d, 128)],
                                             xc[g][:, r],
                                             start=(t == 0),
                                             stop=(t == NT - 1),
                                             perf_mode=DR)
                        # keep the PE p-state ramp alive across the
                        # DMA-paced chunk boundaries
                        if t in fill_after:
                            for _ in range(N_JS):
                                nc.tensor.matmul(junk_s[:, 0:256],
                                                 ones_bf[0:64, :],
                                                 mk[:, 0:256],
                                                 start=True, stop=True,
                                                 skip_group_check=True)
                    # scaled fp8 payload, staged in the sample-interleaved
                    # ReduceScatter chunk layout; two halves so the first
                    # stage DMA overlaps the second casts
                    in_v = in_b[:].rearrange("(c p d) u -> p c (d u)",
                                             c=N_CORES, d=4, p=128)
                    e8r = e8v[:].rearrange("p c d u -> p c (d u)")
                    for dp in range(2):
                        for d in (2 * dp, 2 * dp + 1):
                            src = psum_v[d][:].rearrange(
                                "p (c u) -> p c u", c=N_CORES)
                            if d % 2 == 1:
                                nc.vector.tensor_scalar_mul(
                                    e8v[:, :, d, :], src, SP8)
                            else:
                                nc.scalar.activation(e8v[:, :, d, :], src,
                                                     AF.Copy, scale=SP8)
                        du = slice(dp * 128, dp * 128 + 128)
                        nc.sync.dma_start(out=in_v[:, :, du],
                                          in_=e8r[:, :, du])
                    nc.sync.dma_start(out=xa_sb[:], in_=xar[:])
                    nc.sync.dma_start(out=wa_sb[:], in_=war[:])
                    # the visual ReduceScatter + result load go on the Pool
                    # queue NOW so nothing later delays their dispatch;
                    # er8 loads via DVE-issued DMA so the in-order SP queue
                    # (audio DMAs behind it) is not blocked on RS1
                    nc.gpsimd.collective_compute(
                        "ReduceScatter", ALU.add,
                        replica_groups=[list(range(N_CORES))],
                        ins=[in_b[:]], outs=[rs_b[:]],
                    )
                    er8 = embp.tile([128, 4, 64], F8)
                    nc.sync.dma_start(
                        out=er8[:],
                        in_=rs_b[:].rearrange("(p d) u -> p d u", p=128))
                    # filler batch A: bridge the PE gap between the k-stream
                    # trail and the audio matmuls (dead psum region, mk input)
                    for _ in range(N_JA):
                        nc.tensor.matmul(junk_s[:],
                                         ones_bf[0:64, :], mk[:, 0:512],
                                         start=True, stop=True)

                # ---- audio embed + normalize (hidden under RS1) ----
                e_a = embp.tile([128, 4, S], BF16)
                e_ax = embp.tile([128, 4, 64], BF16)
                with tc.tile_pool(name="pau", bufs=1, space="PSUM") as paup:
                    psum_a = [paup.tile([128, S], F32, tag=f"pa{d}",
                                        name=f"psum_a{d}") for d in range(4)]
                    psum_ax = [paup.tile([128, 64], F32, tag=f"px{d}",
                                         name=f"psum_ax{d}") for d in range(4)]
                    for t in range(NTA):
                        for d in range(4):
                            nc.tensor.matmul(psum_a[d][:],
                                             wa_sb[:, t, :, ts(d, 128)],
                                             xa_sb[:, t, :, 0:S],
                                             start=(t == 0),
                                             stop=(t == NTA - 1),
                                             perf_mode=DR)
                            nc.tensor.matmul(psum_ax[d][:],
                                             wa_sb[:, t, :, ts(d, 128)],
                                             xa_sb[:, t, :, S:SA],
                                             start=(t == 0),
                                             stop=(t == NTA - 1),
                                             perf_mode=DR)
                    for d in range(4):
                        if d < 2:
                            nc.vector.tensor_copy(e_a[:, d], psum_a[d][:])
                            nc.vector.tensor_copy(e_ax[:, d], psum_ax[d][:])
                        else:
                            nc.scalar.copy(e_a[:, d], psum_a[d][:])
                            nc.scalar.copy(e_ax[:, d], psum_ax[d][:])

                with tc.tile_pool(name="paux", bufs=1, space="PSUM") as pauxp:
                    junk_b = pauxp.tile([1, 512], F32, tag="jb")
                    for _ in range(N_JB // 3):
                        nc.tensor.matmul(junk_b[:], ones_bf[0:64, :],
                                         mk[:, 0:512], start=True, stop=True)
                    sq_a = embp.tile([128, 4, SA], BF16)
                    nc.vector.tensor_mul(sq_a[:, :, 0:S], e_a[:], e_a[:])
                    nc.vector.tensor_mul(sq_a[:, :, S:SA], e_ax[:], e_ax[:])
                    psh_a = pauxp.tile([1, SA], F32, tag="psha")
                    for d in range(4):
                        nc.tensor.matmul(psh_a[:, 0:S], ones_bf[:],
                                         sq_a[:, d, 0:S],
                                         start=(d == 0), stop=(d == 3))
                    for d in range(4):
                        nc.tensor.matmul(psh_a[:, S:SA], ones_bf[:],
                                         sq_a[:, d, S:SA],
                                         start=(d == 0), stop=(d == 3))
                    for _ in range(N_JB // 3):
                        nc.tensor.matmul(junk_b[:], ones_bf[0:64, :],
                                         mk[:, 0:512], start=True, stop=True)
                    ln_a = embp.tile([1, SA], BF16)
                    nc.scalar.activation(ln_a[:], psh_a[:], AF.Ln)
                    lnb_a = pauxp.tile([128, S], F32, tag="lnb")
                    nc.tensor.matmul(lnb_a[:], ones_row_bf[:], ln_a[0:1, 0:S],
                                     start=True, stop=True)
                    # rn_a16 = SXN * |e_a|^-1 (joint table: exp(-.5 ln + ln16))
                    rn_a16 = embp.tile([128, S], BF16)
                    nc.scalar.activation(rn_a16[:], lnb_a[:], AF.Exp,
                                         scale=-0.5, bias=bias_pl16[:])
                    for d in range(4):
                        nc.vector.tensor_mul(er_n8[:, d], e_a[:, d],
                                             rn_a16[:])
                    lnb_x = pauxp.tile([128, S], F32, tag="lnb")
                    nc.tensor.matmul(lnb_x[:, 0:64], ones_row_bf[:],
                                     ln_a[0:1, S:SA], start=True, stop=True)
                    rn_x = embp.tile([128, 64], BF16)
                    nc.scalar.activation(rn_x[:], lnb_x[:, 0:64], AF.Exp,
                                         scale=-0.5)
                    er_nx = embp.tile([128, 4, 64], BF16)
                    for d in range(4):
                        nc.vector.tensor_mul(er_nx[:, d], e_ax[:, d], rn_x[:])
                    # local a1*a2 diagonal dots -> [32,1] partition layout
                    tpa = embp.tile([128, 4, 32], BF16)
                    nc.vector.tensor_mul(tpa[:], er_nx[:, :, 0:32],
                                         er_nx[:, :, 32:64])
                    pa6 = pauxp.tile([32, 1], F32, tag="pa6")
                    for d in range(4):
                        nc.tensor.matmul(pa6[:], tpa[:, d, :],
                                         ones_bf[:, 0:1],
                                         start=(d == 0), stop=(d == 3))
                    nc.scalar.activation(a6p[:], pa6[:], AF.Exp)
                    for _ in range(N_JB - 2 * (N_JB // 3)):
                        nc.tensor.matmul(junk_b[:], ones_bf[0:64, :],
                                         mk[:, 0:512], start=True, stop=True)

            # ---------------- local tail on raw chunk embeddings ---------
            if True:
                with tc.tile_pool(name="tail", bufs=1) as tp:
                  with tc.tile_pool(name="pmid", bufs=1, space="PSUM") as pm:
                    # tail-pool filler bridge: keeps the PE engine busy (and
                    # its p-state ramped) through the ReduceScatter + er8
                    # load so the Gram matmuls below run at full clock
                    junk_t = pm.tile([1, 512], F32, tag="jkt")
                    for _ in range(N_JC):
                        nc.tensor.matmul(junk_t[:], ones_bf[0:64, :],
                                         mk[:, 0:512], start=True, stop=True)

                    # contrib columns: {a1-v exps, a2-v exps, v1v2, a1a2}
                    contrib = tp.tile([64, 4], BF16)
                    nc.vector.memset(contrib[:, 3:4], 0.0)
                    nc.vector.tensor_copy(contrib[0:32, 3:4], a6p[:])

                    # local Grams on raw embeddings
                    gvv = pm.tile([64, 64], F32, tag="gvv")
                    for d in range(4):
                        nc.tensor.matmul(gvv[:], er8[:, d, :], er8[:, d, :],
                                         start=(d == 0), stop=(d == 3))
                    gav = pm.tile([64, S], F32, tag="gav")
                    for d in range(4):
                        nc.tensor.matmul(gav[:], er8[:, d, :], er_n8[:, d, :],
                                         start=(d == 0), stop=(d == 3))

                    # norms from diag(G_vv); rn = |E|^-1 / SXN
                    junk_a = tp.tile([64, 64], BF16)
                    nsq = tp.tile([64, 1], F32)
                    nc.vector.tensor_mul(junk_a[:], gvv[:],
                                         mk[:, MK_I64:MK_I64 + 64])
                    nc.vector.reduce_sum(nsq[:], junk_a[:],
                                         axis=mybir.AxisListType.X)
                    # raw v1*v2 pair dots (M32 already carries SXN^2);
                    # emitted before the rn_bm copy so the in-order DVE queue
                    # is not parked behind the rn_f dependency
                    junk_bb = tp.tile([64, 64], BF16)
                    dvv = tp.tile([64, 1], F32)
                    nc.vector.tensor_mul(junk_bb[:], gvv[:],
                                         mk[:, MK_M32:MK_M32 + 64])
                    nc.vector.reduce_sum(dvv[:], junk_bb[:],
                                         axis=mybir.AxisListType.X)
                    lns = tp.tile([64, 1], F32)
                    nc.scalar.activation(lns[:], nsq[:], AF.Ln)
                    rn_f = tp.tile([64, 1], F32)
                    nc.scalar.activation(rn_f[:], lns[:], AF.Exp,
                                         scale=-0.5, bias=bias_nl16[:])
                    rn_bm = tp.tile([64, 1], BF16)
                    nc.vector.tensor_copy(rn_bm[:], rn_f[:])
                    psh_p = pm.tile([64, 1], F32, tag="pshp")
                    nc.tensor.matmul(psh_p[:], mk[:, MK_PSH:MK_PSH + 64],
                                     rn_bm[:], start=True, stop=True)
                    rnp12 = tp.tile([64, 1], F32)
                    nc.vector.tensor_mul(rnp12[:], rn_f[:], psh_p[:])

                    # exp Gram with normalization folded into the act scale
                    exp_gt = tp.tile([64, S], BF16)
                    junk_c = tp.tile([64, S], BF16)
                    denp = pm.tile([1, 256], F32, tag="denp")
                    nc.scalar.activation(exp_gt[:], gav[:], AF.Exp,
                                         scale=rn_f[:])
                    nc.scalar.activation(contrib[:, 2:3], dvv[:], AF.Exp,
                                         scale=rnp12[:], bias=bias_n30[:])
                    nc.vector.tensor_mul(junk_c[:], exp_gt[:],
                                         mk[:, MK_MAV:MK_MAV + S])
                    nc.tensor.matmul(denp[:], ones_bf[0:64, :],
                                     exp_gt[:, 0:256], start=True, stop=False)
                    nc.tensor.matmul(denp[:], ones_bf[0:64, :],
                                     exp_gt[:, 256:512], start=False,
                                     stop=True)
                    with nc.allow_low_precision(
                            reason="sum of exps in bf16; tolerance 2e-2"):
                        nc.vector.reduce_sum(
                            contrib[:, 0:2],
                            junk_c[:].rearrange("p (h n) -> p h n", h=2),
                            axis=mybir.AxisListType.X)

                    # fold rows j/j+32; place num into global batch cols
                    selp = pm.tile([32, 4], F32, tag="selp")
                    nc.tensor.matmul(selp[:], mk[:, MK_SEL:MK_SEL + 32],
                                     contrib[:], start=True, stop=True)
                    nsum = tp.tile([32, 1], F32)
                    nc.vector.reduce_sum(nsum[:], selp[:],
                                         axis=mybir.AxisListType.X)
                    recip = tp.tile([32, 1], BF16)
                    with nc.allow_low_precision(
                            reason="1/num in bf16; tolerance 2e-2"):
                        nc.vector.reciprocal(recip[:], nsum[:])
                    placep = pm.tile([1, 256], F32, tag="plcp")
                    nc.tensor.matmul(placep[:], recip[:],
                                     mk[0:32, MK_PLC:MK_PLC + 256],
                                     start=True, stop=True)

                    # payload = [den partial (256) | num in our slot (256)];
                    # stage 8 replicas with one broadcast-read DMA
                    payload = tp.tile([1, 512], BF16)
                    nc.scalar.copy(payload[:, 0:256], denp[:])
                    nc.vector.tensor_copy(payload[:, 256:512], placep[:])
                    nc.sync.dma_start(
                        out=rs2_in[:].rearrange("(o a) b -> o (a b)", o=1),
                        in_=payload[:].rearrange(
                            "p (o n) -> p o n", o=1).to_broadcast((1, 8, 512)))

                  # (pmid closed: the final reduction gets its own psum)
                  if True:
                    # replicated-input ReduceScatter == cheap AllReduce:
                    # every core receives [sum-den(256) | all num(256)]
                    nc.gpsimd.collective_compute(
                        "ReduceScatter", ALU.add,
                        replica_groups=[list(range(N_CORES))],
                        ins=[rs2_in[:]], outs=[rs2_out[:]],
                    )
                    g2 = tp.tile([1, 512], BF16)
                    nc.sync.dma_start(out=g2[:], in_=rs2_out[:].opt())
                    # one Ln over [den | 1/num]: accum = sum ln den - sum ln
                    # num = 256 * loss
                    l_all = tp.tile([1, 512], F32)
                    dsum = tp.tile([1, 1], F32)
                    nc.scalar.activation(l_all[:], g2[:], AF.Ln,
                                         accum_out=dsum[:])
                    loss_sb = tp.tile([1, 1], F32)
                    nc.scalar.activation(loss_sb[:], dsum[:], AF.Copy,
                                         scale=float(1.0 / B))
                    nc.sync.dma_start(out=loss_d.ap(), in_=loss_sb[:])

    nc.compile()
    return nc


def _get_nc():
    if "nc" not in _CACHE:
        _CACHE["nc"] = build()
    return _CACHE["nc"]


def _dr_layout(m, nt):
    """[nt*256, N] k-major -> [128, nt*2*N] DoubleRow DMA layout.
    Logical k = t*256 + i*128 + p lands at [p, t, i, :]."""
    n = m.shape[1]
    return np.ascontiguousarray(
        m.reshape(nt, 2, 128, n).transpose(2, 0, 1, 3)).reshape(128, nt * 2 * n)


def _vperm():
    """Permuted visual sample order: chunk c = [v1 batch 32c..32c+32,
    v2 batch 32c..32c+32]; v2 originals live at sample index 256+i."""
    perm = []
    for c in range(N_CORES):
        perm.extend(range(32 * c, 32 * c + 32))
        perm.extend(range(256 + 32 * c, 256 + 32 * c + 32))
    return np.asarray(perm)


def _masks(c):
    """Per-core [64, MK_N] mask constants (see MK_* layout)."""
    mk = np.zeros((64, MK_N), np.float32)
    r = np.arange(64)
    mk[r, MK_I64 + r] = 1.0                        # identity
    s = np.arange(64)
    mk[(s + 32) % 64, MK_PSH + s] = 1.0            # +32 shift permutation
    j = np.arange(32)
    mk[j, MK_M32 + 32 + j] = SXN * SXN             # v1-v2 pair mask
    mk[j, MK_SEL + j] = 1.0                        # fold rows j, j+32
    mk[j + 32, MK_SEL + j] = 1.0
    mk[j, MK_PLC + 32 * c + j] = 1.0               # place into global cols
    col = 32 * c + (r % 32)
    mk[r, MK_MAV + col] = 1.0                      # a1 block
    mk[r, MK_MAV + 256 + col] = 1.0                # a2 block
    return mk.astype(ml_dtypes.bfloat16)


def _shard_inputs(a_1, v_1, a_2, v_2, W_a, W_v):
    f8 = ml_dtypes.float8_e4m3
    A = np.concatenate([a_1, a_2], axis=0).reshape(S, KA)
    V = np.concatenate([v_1, v_2], axis=0)
    V = V.reshape(S, 15, 96, 96)[:, :, 48:, :].reshape(S, KV_TOT)
    Wvp = np.ascontiguousarray(
        W_v.reshape(5, 3, 48 * 96, D).transpose(1, 0, 2, 3)
    ).reshape(KV_TOT, D)

    A8 = (A.T * SX).astype(f8)                 # (1280, 512)
    V8 = (V.T * SX).astype(f8)[:, _vperm()]    # (69120, 512) permuted cols
    Wa8 = (W_a * SW).astype(f8)
    Wv8 = (Wvp * SW).astype(f8)

    wa = _dr_layout(np.ascontiguousarray(Wa8), NTA)

    in_maps = []
    for c in range(N_CORES):
        xv = np.zeros((KVP, S), f8)
        xv[:KV] = V8[c * KV:(c + 1) * KV]
        wv = np.zeros((KVP, D), f8)
        wv[:KV] = Wv8[c * KV:(c + 1) * KV]
        # canonical audio + this core's 64 pair columns (a1 then a2)
        ec = list(range(32 * c, 32 * c + 32)) + \
             list(range(256 + 32 * c, 256 + 32 * c + 32))
        xa_c = np.concatenate([A8, A8[:, ec]], axis=1)   # (1280, 576)
        in_maps.append({
            "xv": _dr_layout(xv, NT),
            "wv": _dr_layout(wv, NT),
            "xa": _dr_layout(np.ascontiguousarray(xa_c), NTA),
            "wa": wa,
            "mk": _masks(c),
        })
    return in_maps


def kernel(a_1, v_1, a_2, v_2, W_a, W_v):
    nc = _get_nc()
    in_maps = _shard_inputs(np.asarray(a_1, np.float32),
                            np.asarray(v_1, np.float32),
                            np.asarray(a_2, np.float32),
                            np.asarray(v_2, np.float32),
                            np.asarray(W_a, np.float32),
                            np.asarray(W_v, np.float32))
    res = bass_utils.run_bass_kernel_spmd(nc, in_maps,
                                          core_ids=list(range(N_CORES)))
    return np.asarray(res.results[0]["loss"], np.float32).reshape(())
